# revision 10
# baseline (speedup 1.0000x reference)
"""Convolutional self-attention (SAGAN-style) Trainium2 Bass kernel.

Math (per batch element, X = skip[b] reshaped [C, N], Xr = res[b]):
    f  = Wq @ X              [D, N]
    g  = Wk @ X              [D, N]
    hv = Wv @ Xr             [C, N]
    s[i, j]   = sum_d f[d, i] g[d, j]
    beta[:, j] = softmax(s[:, j])        (softmax over i)
    out = gamma * hv @ beta + X

Sharding: data-parallel over batch B=8, one batch element per NeuronCore.
Weights are replicated (tiny). gamma is folded into Wv on the host.

Device strategy (per core):
    - all matmuls in float32r (fp32 bits streamed as FP22: full PE rate at
      N=512 moving dim, ~1e-4 relative error)
    - scores computed in [i, j] layout so exp(s) tiles feed the output
      matmul directly as the moving operand (contraction over i)
    - softmax denominator via a ones-vector matmul accumulated in PSUM
      (partition-axis sum on the PE); no max subtraction needed since
      |s| <~ 15 (inputs are unit-normal with 0.02-scaled weights)
    - 1/S broadcast across partitions via a rank-1 PE outer product
    - j-blocked: PSUM holds 4 accumulator banks (C=512 rows) + 2 score
      banks + 2 sum banks = 8 banks exactly
"""

from contextlib import ExitStack

import numpy as np

import concourse.bacc as bacc
import concourse.bass as bass
import concourse.mybir as mybir
import concourse.tile as tile
from concourse.bass import ts
from concourse.bass_utils import run_bass_kernel_spmd

B, C, HH, WW = 8, 512, 64, 64
N = HH * WW  # 4096
D = 64       # query/key channels (C // 8)
P = 128      # SBUF/PSUM partitions
CC = C // P  # 4 channel chunks
NI = N // P  # 32 i-chunks
JW = 512     # j-block width (one PSUM bank of fp32)
NJ = N // JW  # 8 j-blocks

F32 = mybir.dt.float32
F32R = mybir.dt.float32r
EXP = mybir.ActivationFunctionType.Exp


def _build():
    nc = bacc.Bacc("TRN2", target_bir_lowering=False, debug=False, num_devices=B)

    x_d = nc.dram_tensor("x", [C, N], F32, kind="ExternalInput").ap()
    xr_d = nc.dram_tensor("xr", [C, N], F32, kind="ExternalInput").ap()
    wq_d = nc.dram_tensor("wqt", [C, D], F32, kind="ExternalInput").ap()
    wk_d = nc.dram_tensor("wkt", [C, D], F32, kind="ExternalInput").ap()
    wv_d = nc.dram_tensor("wvt", [C, C], F32, kind="ExternalInput").ap()
    ones_d = nc.dram_tensor("ones", [P, P], F32, kind="ExternalInput").ap()
    o_d = nc.dram_tensor("o", [C, N], F32, kind="ExternalOutput").ap()

    with tile.TileContext(nc) as tc, ExitStack() as ctx:
        const = ctx.enter_context(tc.tile_pool(name="const", bufs=1))
        big = ctx.enter_context(tc.tile_pool(name="big", bufs=1))
        xrp = ctx.enter_context(tc.tile_pool(name="xrp", bufs=4))
        expp = ctx.enter_context(tc.tile_pool(name="expp", bufs=3))
        bcp = ctx.enter_context(tc.tile_pool(name="bcp", bufs=2))
        outp = ctx.enter_context(tc.tile_pool(name="outp", bufs=3))
        recp = ctx.enter_context(tc.tile_pool(name="recp", bufs=2))
        ps_o = ctx.enter_context(
            tc.tile_pool(name="ps_o", bufs=CC, space=bass.MemorySpace.PSUM)
        )
        ps_sc = ctx.enter_context(
            tc.tile_pool(name="ps_sc", bufs=2, space=bass.MemorySpace.PSUM)
        )
        ps_s = ctx.enter_context(
            tc.tile_pool(name="ps_s", bufs=2, space=bass.MemorySpace.PSUM)
        )

        # ---- resident inputs ----
        x_sb = big.tile([P, CC, N], F32R, tag="x_sb")
        nc.sync.dma_start(
            out=x_sb[:], in_=x_d.rearrange("(cc p) n -> p cc n", p=P).bitcast(F32R)
        )
        wq_sb = const.tile([P, CC, D], F32R, tag="wq")
        nc.sync.dma_start(
            out=wq_sb[:], in_=wq_d.rearrange("(cc p) d -> p cc d", p=P).bitcast(F32R)
        )
        wk_sb = const.tile([P, CC, D], F32R, tag="wk")
        nc.sync.dma_start(
            out=wk_sb[:], in_=wk_d.rearrange("(cc p) d -> p cc d", p=P).bitcast(F32R)
        )
        wv_sb = const.tile([P, CC, C], F32R, tag="wv")
        nc.sync.dma_start(
            out=wv_sb[:], in_=wv_d.rearrange("(cc p) c -> p cc c", p=P).bitcast(F32R)
        )
        ones_sb = const.tile([P, P], F32R, tag="ones_sb")
        nc.sync.dma_start(out=ones_sb[:], in_=ones_d.bitcast(F32R))
        ones_col = ones_sb[:, 0:1]
        ones_row = ones_sb[0:1, :]

        # ---- f = Wq X, g = Wk X (contract over c in 4 chunks) ----
        f_sb = big.tile([D, N], F32R, tag="f_sb")
        g_sb = big.tile([D, N], F32R, tag="g_sb")
        for w_sb, t_sb in ((wq_sb, f_sb), (wk_sb, g_sb)):
            for nb in range(N // JW):
                mm = ps_sc.tile([D, JW], F32, tag="sc")
                for cc in range(CC):
                    nc.tensor.matmul(
                        mm[:],
                        w_sb[:, cc, :],
                        x_sb[:, cc, ts(nb, JW)],
                        start=(cc == 0),
                        stop=(cc == CC - 1),
                    )
                nc.scalar.copy(out=t_sb[:, ts(nb, JW)], in_=mm[:])

        # ---- hvT[i, c] = sum_c' Xr[c', i] WvT[c', c] (gamma pre-folded) ----
        hv_sb = big.tile([P, NI, C], F32R, tag="hv_sb")
        for ic in range(NI):
            mm = ps_sc.tile([P, C], F32, tag="sc")
            for cc in range(CC):
                xt = xrp.tile([P, P], F32R, tag="xr")
                nc.sync.dma_start(
                    out=xt[:],
                    in_=xr_d[cc * P : (cc + 1) * P, ts(ic, P)].bitcast(F32R),
                )
                nc.tensor.matmul(
                    mm[:],
                    xt[:],
                    wv_sb[:, cc, :],
                    start=(cc == 0),
                    stop=(cc == CC - 1),
                )
            nc.scalar.copy(out=hv_sb[:, ic, :], in_=mm[:])

        # ---- attention main loop, j-blocked ----
        def emit_scores(jb, ic):
            scp = ps_sc.tile([P, JW], F32, tag="sc")
            nc.tensor.matmul(
                scp[:],
                f_sb[:, ts(ic, P)],
                g_sb[:, ts(jb, JW)],
                start=True,
                stop=True,
            )
            et = expp.tile([P, JW], F32R, tag="exp")
            nc.scalar.activation(out=et[:], in_=scp[:], func=EXP)
            return et

        def make_epilogue(jb, o_ps, s_ps):
            def epi():
                rc = recp.tile([1, JW], F32R, tag="rec")
                with nc.allow_low_precision(
                    reason="1/S feeds an fp32r matmul; fp22 is plenty here"
                ):
                    nc.vector.reciprocal(out=rc[:], in_=s_ps[:])
                bcps = ps_sc.tile([P, JW], F32, tag="sc")
                nc.tensor.matmul(
                    bcps[:], ones_row, rc[:], start=True, stop=True
                )
                bc = bcp.tile([P, JW], F32, tag="bc")
                nc.vector.tensor_copy(out=bc[:], in_=bcps[:])
                for cc in range(CC):
                    ot = outp.tile([P, JW], F32, tag="out")
                    nc.vector.tensor_mul(out=ot[:], in0=o_ps[cc][:], in1=bc[:])
                    nc.vector.tensor_add(
                        out=ot[:], in0=ot[:], in1=x_sb[:, cc, ts(jb, JW)].bitcast(F32)
                    )
                    nc.sync.dma_start(
                        out=o_d[cc * P : (cc + 1) * P, ts(jb, JW)], in_=ot[:]
                    )

            return epi

        pending = None
        for jb in range(NJ):
            o_ps = [
                ps_o.tile([P, JW], F32, tag="o", name=f"o_ps_{jb}_{cc}")
                for cc in range(CC)
            ]
            s_ps = ps_s.tile([1, JW], F32, tag="S")
            e_next = emit_scores(jb, 0)
            for ic in range(NI):
                et = e_next
                if ic + 1 < NI:
                    e_next = emit_scores(jb, ic + 1)
                if ic == 0 and pending is not None:
                    # previous j-block's epilogue, emitted here so its PE
                    # work hides the reciprocal latency behind fresh scores
                    pending()
                    pending = None
                nc.tensor.matmul(
                    s_ps[:],
                    ones_col,
                    et[:],
                    start=(ic == 0),
                    stop=(ic == NI - 1),
                )
                for cc in range(CC):
                    nc.tensor.matmul(
                        o_ps[cc][:],
                        hv_sb[:, ic, ts(cc, P)],
                        et[:],
                        start=(ic == 0),
                        stop=(ic == NI - 1),
                    )
            pending = make_epilogue(jb, o_ps, s_ps)
        pending()

    nc.compile()
    return nc


_CACHE = {}


def _get_nc():
    if "nc" not in _CACHE:
        _CACHE["nc"] = _build()
    return _CACHE["nc"]


def kernel(skip, res, Wq, Wk, Wv, gamma):
    nc = _get_nc()
    skip = np.ascontiguousarray(np.asarray(skip, dtype=np.float32)).reshape(B, C, N)
    res_ = np.ascontiguousarray(np.asarray(res, dtype=np.float32)).reshape(B, C, N)
    gamma_v = float(np.asarray(gamma, dtype=np.float32).reshape(-1)[0])
    wqT = np.ascontiguousarray(np.asarray(Wq, dtype=np.float32).T)
    wkT = np.ascontiguousarray(np.asarray(Wk, dtype=np.float32).T)
    wvT = np.ascontiguousarray((gamma_v * np.asarray(Wv, dtype=np.float32)).T)

    ones = np.ones((P, P), dtype=np.float32)
    in_maps = [
        {
            "x": np.ascontiguousarray(skip[b]),
            "xr": np.ascontiguousarray(res_[b]),
            "wqt": wqT,
            "wkt": wkT,
            "wvt": wvT,
            "ones": ones,
        }
        for b in range(B)
    ]
    out = run_bass_kernel_spmd(
        nc, in_maps, core_ids=list(range(B)), trace=_CACHE.get("trace", False)
    )
    _CACHE["last"] = out
    o = np.stack([r["o"] for r in out.results], axis=0)
    return o.reshape(B, C, HH, WW).astype(np.float32, copy=False)


if __name__ == "__main__":
    rng = np.random.default_rng(0)
    ins = {
        "skip": rng.standard_normal((B, C, HH, WW), dtype=np.float32),
        "res": rng.standard_normal((B, C, HH, WW), dtype=np.float32),
        "Wq": (rng.standard_normal((D, C)) * 0.02).astype(np.float32),
        "Wk": (rng.standard_normal((D, C)) * 0.02).astype(np.float32),
        "Wv": (rng.standard_normal((C, C)) * 0.02).astype(np.float32),
        "gamma": np.asarray([0.5], dtype=np.float32),
    }
    o = kernel(**ins)
    print(o.shape, o.dtype)
